# revision 3
# baseline (speedup 1.0000x reference)
"""Trainium2 Bass kernel for nn_CognitiveManifold (geodesic RK2 step).

8 NeuronCores, pure data parallel: 8192 tokens/core, full inputs in,
full outputs out. Analytic metric derivatives (matches the reference's
eps=1e-4 central FD to ~1e-8) + one 8x8 SPD LDL^T solve per token.

Per-chunk layouts (TC=4096 tokens, token_local = 32*p + q):
  A' (tokens on partitions): [128, (q=32, feat)]
  B  (features on partitions, tokens on free), via PE transpose:
    (d)-space  [64  = 8*q3+d,   (H=4, 128p)]    q = 8H + q3
    (j)-space  [128 = 16*q3+j,  (H=4, 128p)]
    (mn)-space [128 = 64*qs+mn, (P=16, 128p)]   q = 2P + qs
"""

import numpy as np

try:  # concourse ships with the container; ensure it's importable
    import concourse  # noqa: F401
except ImportError:  # pragma: no cover
    import sys as _sys
    for _p in ("/opt/trn_rl_repo", "/root/.axon_site/_ro/trn_rl_repo"):
        if _p not in _sys.path:
            _sys.path.insert(0, _p)

LAST_EXEC_TIME_NS = None
LAST_TRACE_PATH = None
D = 8
NCORES = 8
NTOK = 8192
TC = 4096
NCHUNK = NTOK // TC
NP = 128
NQ = TC // NP      # 32


def _build_consts(L, W1, b1, W2, b2, Wr1, br1, Wr2, br2):
    f = np.float32
    L, W1, b1, W2, b2 = (np.asarray(a, np.float64) for a in (L, W1, b1, W2, b2))
    Wr1, br1, Wr2, br2 = (np.asarray(a, np.float64) for a in (Wr1, br1, Wr2, br2))
    G0 = L @ L.T + 1e-4 * np.eye(D)
    W2r = W2.reshape(16, D, D)
    W2sym = (0.5 * (W2r + np.swapaxes(W2r, 1, 2))).reshape(16, 64)
    b2r = b2.reshape(D, D)
    b2sym = (0.5 * (b2r + b2r.T)).reshape(64)
    W2sym2 = (W2r + np.swapaxes(W2r, 1, 2)).reshape(16, 64)
    Wdr0 = Wr1 * Wr2[:, 0][None, :]          # [r, j] = Wr1[r,j]*Wr2[j,0]

    def blockdiag(w, g):
        kin, mout = w.shape
        out = np.zeros((g * kin, g * mout), dtype=np.float64)
        for i in range(g):
            out[i * kin:(i + 1) * kin, i * mout:(i + 1) * mout] = w
        return out

    C = {}
    C["eye128"] = np.eye(128)
    C["ones1"] = np.ones((1, 128))
    g0row = np.zeros((1, 128))
    g0row[0, :64] = (10.0 * G0).reshape(64)
    g0row[0, 64:] = (10.0 * G0).reshape(64)
    C["g0row10"] = g0row
    C["bd_w1"] = blockdiag(W1, 8)                 # [64,128]
    C["bd_wr1"] = blockdiag(Wr1, 8)               # [64,64]
    C["bd_g0"] = blockdiag(G0, 8)                 # [64,64]
    # per-Pl expanded (q3-selective) weights: [(q3,j),(qs,mn)] = d_{q3,2Pl+qs}*W
    for Pl in range(4):
        w = np.zeros((128, 128))
        w2 = np.zeros((128, 128))
        for qs in range(2):
            q3 = 2 * Pl + qs
            w[q3 * 16:(q3 + 1) * 16, qs * 64:(qs + 1) * 64] = W2sym
            w2[q3 * 16:(q3 + 1) * 16, qs * 64:(qs + 1) * 64] = W2sym2
        C[f"bd_w2sym_{Pl}"] = w
        C[f"bd_w2sym2_{Pl}"] = w2
    C["bd_w2q"] = blockdiag(0.1 * W2.T, 2)        # [128,32]
    sel = np.zeros((128, 16))
    for qs in range(2):
        for n in range(D):
            for r in range(D):
                sel[qs * 64 + n * D + r, qs * D + r] = 1.0
    C["selc"] = sel
    ones2 = np.zeros((128, 2))
    ones2[:64, 0] = 1.0
    ones2[64:, 1] = 1.0
    C["onesc"] = ones2
    ones8 = np.zeros((64, 8))
    for q3 in range(8):
        ones8[q3 * D:(q3 + 1) * D, q3] = 1.0
    C["ones8c"] = ones8
    C["w1tc"] = blockdiag(W1.T, 8)                # [128,64]
    C["wdr0c"] = blockdiag(Wdr0.T, 8)             # [64,64]
    wr2c = np.zeros((64, 8))
    for q3 in range(8):
        wr2c[q3 * D:(q3 + 1) * D, q3] = Wr2[:, 0]
    C["wr2c"] = wr2c
    # repXc_Pl [64,128]: [(q3,d),(qs,(n,r))] = d_{q3,2Pl+qs} * d_{d,n or d,r}
    for Pl in range(4):
        rep1 = np.zeros((64, 128))
        rep2 = np.zeros((64, 128))
        for qs in range(2):
            q3 = 2 * Pl + qs
            for d in range(D):
                for r in range(D):
                    rep1[q3 * D + d, qs * 64 + d * D + r] = 1.0   # n = d
                    rep2[q3 * D + d, qs * 64 + r * D + d] = 1.0   # r = d
        C[f"rep1c_{Pl}"] = rep1
        C[f"rep2c_{Pl}"] = rep2
    C["b1c"] = np.tile(b1, 8)                     # [128]
    C["br1c"] = np.tile(br1, 8)                   # [64]
    C["b2symc"] = np.tile(b2sym, 2)               # [128]
    C = {k: np.ascontiguousarray(v, dtype=f) for k, v in C.items()}
    return C, float(br2[0])


CONST_SHAPES = {
    "eye128": (128, 128), "ones1": (1, 128), "g0row10": (1, 128),
    "bd_w1": (64, 128), "bd_wr1": (64, 64), "bd_g0": (64, 64),
    "bd_w2q": (128, 32),
    "selc": (128, 16), "onesc": (128, 2), "ones8c": (64, 8),
    "w1tc": (128, 64), "wdr0c": (64, 64), "wr2c": (64, 8),
    "b1c": (128,), "br1c": (64,), "b2symc": (128,),
    **{f"bd_w2sym_{p}": (128, 128) for p in range(4)},
    **{f"bd_w2sym2_{p}": (128, 128) for p in range(4)},
    **{f"rep1c_{p}": (64, 128) for p in range(4)},
    **{f"rep2c_{p}": (64, 128) for p in range(4)},
}


def _emit(nc, tc, ctx, dram, br2f):
    import concourse.mybir as mybir

    f32 = mybir.dt.float32
    AF = mybir.ActivationFunctionType
    OP = mybir.AluOpType

    consts = ctx.enter_context(tc.tile_pool(name="consts", bufs=1))
    sb = ctx.enter_context(tc.tile_pool(name="sb", bufs=2))
    sbig = ctx.enter_context(tc.tile_pool(name="sbig", bufs=1))
    sbA = ctx.enter_context(tc.tile_pool(name="sbA", bufs=2))
    wps = ctx.enter_context(tc.tile_pool(name="wps", bufs=3, space="PSUM"))
    gtps = ctx.enter_context(tc.tile_pool(name="gtps", bufs=1, space="PSUM"))
    scps = ctx.enter_context(tc.tile_pool(name="scps", bufs=1, space="PSUM"))

    cs = {}
    for name, shape in CONST_SHAPES.items():
        if len(shape) == 1:
            t = consts.tile([shape[0], 1], f32, name=name, tag=name)
            nc.sync.dma_start(out=t[:, :],
                              in_=dram[name].rearrange("(p one) -> p one", one=1))
        else:
            t = consts.tile(list(shape), f32, name=name, tag=name)
            nc.sync.dma_start(out=t[:, :], in_=dram[name][:, :])
        cs[name] = t
    ident = cs["eye128"]
    br2t = consts.tile([128, 1], f32, name="br2t")
    nc.vector.memset(br2t[:, :], br2f)
    br2h = consts.tile([128, 1], f32, name="br2h")
    nc.vector.memset(br2h[:, :], 0.5 * br2f)
    onet = consts.tile([128, 1], f32, name="onet")
    nc.vector.memset(onet[:, :], 1.0)

    def dram_chunk(t, c):
        return t[c * TC:(c + 1) * TC, :].rearrange("(p q) d -> p (q d)", q=NQ)

    def transpose2(src, tag):
        """[128, 256] A'-(q,d) -> B (d)-space [64 = 8*q3+d, (H4, 128p)] in SBUF."""
        out = sb.tile([64, 512], f32, tag=tag)
        pt = wps.tile([128, 512], f32, tag="work_ps")
        for H in range(4):
            nc.tensor.matmul(pt[:64, H * 128:(H + 1) * 128],
                             src[:, H * 64:(H + 1) * 64],
                             ident[:, :], is_transpose=True, start=True, stop=True)
        nc.vector.tensor_copy(out[:, :], pt[:64, :])
        return out

    def emit_call(xT, vT, vA, aA):
        """One christoffel+contraction; writes acceleration into aA [128,(q,8)]."""

        # ---------- forward matmuls ((j)/(d)-space) ----------
        u_ps = wps.tile([128, 512], f32, tag="work_ps")
        s_ps = wps.tile([128, 512], f32, tag="work_ps")
        for H in range(4):
            rhs = xT[:, H * 128:(H + 1) * 128]
            sl = slice(H * 128, (H + 1) * 128)
            nc.tensor.matmul(u_ps[:, sl], cs["bd_w1"][:, :], rhs, start=True, stop=True)
            nc.tensor.matmul(s_ps[:64, sl], cs["bd_wr1"][:, :], rhs, start=True, stop=True)
        a1B = sb.tile([128, 512], f32, tag="a1B")
        gpuB = sb.tile([128, 512], f32, tag="gpuB")
        nc.scalar.activation(a1B[:, :], u_ps[:, :], AF.Gelu, bias=cs["b1c"][:, :])
        nc.scalar.activation(gpuB[:, :], u_ps[:, :], AF.Derivative_Gelu,
                             bias=cs["b1c"][:, :])
        a2B = sb.tile([64, 512], f32, tag="a2B")
        gpsB = sb.tile([64, 512], f32, tag="gpsB")
        nc.scalar.activation(a2B[:, :], s_ps[:64, :], AF.Gelu, bias=cs["br1c"][:, :])
        nc.scalar.activation(gpsB[:, :], s_ps[:64, :], AF.Derivative_Gelu,
                             bias=cs["br1c"][:, :])

        c_ps = wps.tile([128, 512], f32, tag="work_ps")
        gv_ps = wps.tile([128, 512], f32, tag="work_ps")
        for H in range(4):
            rhv = vT[:, H * 128:(H + 1) * 128]
            sl = slice(H * 128, (H + 1) * 128)
            nc.tensor.matmul(c_ps[:, sl], cs["bd_w1"][:, :], rhv, start=True, stop=True)
            nc.tensor.matmul(gv_ps[:64, sl], cs["bd_g0"][:, :], rhv, start=True, stop=True)
        cgB = sb.tile([128, 512], f32, tag="cgB")
        nc.vector.tensor_mul(cgB[:, :], c_ps[:, :], gpuB[:, :])
        m1B = sb.tile([64, 512], f32, tag="m1B")
        nc.vector.tensor_mul(m1B[:, :], gv_ps[:64, :], vT[:, :])

        # ---------- scalar-channel + small A'-folds (PSUM pack) ----------
        # pack: [0:32) t | [32:64) QG | [64:96) QE | [96:128) unused
        # [128:384) dr0 | [384:640) T1E | [640:896) T2E
        pk = scps.tile([128, 1024], f32, tag="pack_ps")
        t_ps = pk[:, 0:32]
        qg_ps = pk[:, 32:64]
        qe_ps = pk[:, 64:96]
        dr0_ps = pk[:, 128:384]
        t1e_ps = pk[:, 384:640]
        t2e_ps = pk[:, 640:896]
        for H in range(4):
            hsl = slice(H * 128, (H + 1) * 128)
            nc.tensor.matmul(t_ps[:, H * 8:(H + 1) * 8], a2B[:, hsl],
                             cs["wr2c"][:, :], start=True, stop=True)
            nc.tensor.matmul(qg_ps[:, H * 8:(H + 1) * 8], m1B[:, hsl],
                             cs["ones8c"][:, :], start=True, stop=True)
            nc.tensor.matmul(dr0_ps[:, H * 64:(H + 1) * 64], gpsB[:, hsl],
                             cs["wdr0c"][:, :], start=True, stop=True)

        def stile(tag):
            return sbA.tile([128, 32], f32, tag=tag, name=tag)
        rrawA, sigA, rA, rinvA, kapA, tmpA, uA, absA = (
            stile(t) for t in ["rrawA", "sigA", "rA", "rinvA", "kapA", "tmpA",
                               "uA", "absA"])
        # u = t + br2; softplus(u) = ln(exp(-|u|) + 1) + relu(u)
        nc.scalar.activation(uA[:, :], t_ps[:, :], AF.Identity, bias=br2t[:, :])
        nc.scalar.activation(absA[:, :], t_ps[:, :], AF.Abs, bias=br2t[:, :])
        nc.scalar.activation(absA[:, :], absA[:, :], AF.Exp, scale=-1.0)
        nc.scalar.activation(absA[:, :], absA[:, :], AF.Ln, bias=onet[:, :])
        nc.vector.tensor_scalar_max(rrawA[:, :], uA[:, :], 0.0)
        nc.vector.tensor_add(rrawA[:, :], rrawA[:, :], absA[:, :])
        # sigmoid(u) = 0.5 + 0.5*tanh(u/2)
        nc.scalar.activation(sigA[:, :], t_ps[:, :], AF.Tanh, scale=0.5,
                             bias=br2h[:, :])
        nc.vector.tensor_scalar(out=sigA[:, :], in0=sigA[:, :], scalar1=0.5,
                                scalar2=0.5, op0=OP.mult, op1=OP.add)
        nc.vector.tensor_scalar_max(rA[:, :], rrawA[:, :], 0.1)
        nc.vector.tensor_scalar_min(rA[:, :], rA[:, :], 10.0)
        nc.vector.reciprocal(rinvA[:, :], rA[:, :])
        nc.vector.tensor_scalar(out=kapA[:, :], in0=rrawA[:, :], scalar1=0.1,
                                scalar2=None, op0=OP.is_gt)
        nc.vector.tensor_scalar(out=tmpA[:, :], in0=rrawA[:, :], scalar1=10.0,
                                scalar2=None, op0=OP.is_lt)
        nc.vector.tensor_mul(kapA[:, :], kapA[:, :], tmpA[:, :])
        nc.vector.tensor_mul(kapA[:, :], kapA[:, :], sigA[:, :])

        # ---------- (mn)-space stream ----------
        tanhSB = sbig.tile([128, 2048], f32, tag="tanhSB")
        tanhpB = sbig.tile([128, 2048], f32, tag="tanhpB")
        wtB = sbig.tile([128, 2048], f32, tag="wtB")
        vr1B = sbig.tile([128, 2048], f32, tag="vr1B")
        vvTB = sbig.tile([128, 2048], f32, tag="vvTB")
        ppB = sbig.tile([128, 2048], f32, tag="ppB")
        qqB = sbig.tile([128, 2048], f32, tag="qqB")
        t1preB = sbig.tile([128, 2048], f32, tag="t1preB")
        gA = sbig.tile([128, 2048], f32, tag="gA")
        invdA = sbA.tile([128, 256], f32, tag="invdA")
        wcolA = sbA.tile([128, 224], f32, tag="wcolA")      # (q32, 7)
        tscrA = sbA.tile([128, 1568], f32, tag="tscrA")     # (q32, 49)
        qa_ps = wps.tile([64, 512], f32, tag="qa_ps", bufs=1)
        qb_ps = wps.tile([64, 512], f32, tag="qb_ps", bufs=1)

        for H in range(4):
            hsl = slice(H * 512, (H + 1) * 512)
            S_ps = wps.tile([128, 512], f32, tag="work_ps")
            bs_ps = wps.tile([128, 512], f32, tag="work_ps")
            v1_ps = wps.tile([128, 512], f32, tag="work_ps")
            v2_ps = wps.tile([128, 512], f32, tag="work_ps")
            hb = slice(H * 128, (H + 1) * 128)
            for Pl in range(4):
                psl = slice(Pl * 128, (Pl + 1) * 128)
                nc.tensor.matmul(S_ps[:, psl], cs[f"bd_w2sym_{Pl}"][:, :],
                                 a1B[:, hb], start=True, stop=True)
                nc.tensor.matmul(bs_ps[:, psl], cs[f"bd_w2sym2_{Pl}"][:, :],
                                 cgB[:, hb], start=True, stop=True)
                nc.tensor.matmul(v1_ps[:, psl], cs[f"rep1c_{Pl}"][:, :],
                                 vT[:, hb], start=True, stop=True)
                nc.tensor.matmul(v2_ps[:, psl], cs[f"rep2c_{Pl}"][:, :],
                                 vT[:, hb], start=True, stop=True)
            nc.scalar.activation(tanhSB[:, hsl], S_ps[:, :], AF.Tanh,
                                 bias=cs["b2symc"][:, :])
            nc.scalar.activation(tanhpB[:, hsl], tanhSB[:, hsl], AF.Square)
            nc.scalar.activation(tanhpB[:, hsl], tanhpB[:, hsl], AF.Identity,
                                 scale=-1.0, bias=onet[:, :])
            nc.vector.tensor_copy(vr1B[:, hsl], v1_ps[:, :])
            nc.vector.tensor_mul(vvTB[:, hsl], vr1B[:, hsl], v2_ps[:, :])
            nc.vector.tensor_mul(wtB[:, hsl], tanhpB[:, hsl], bs_ps[:, :])
            nc.vector.tensor_mul(t1preB[:, hsl], wtB[:, hsl], vr1B[:, hsl])
            nc.gpsimd.tensor_mul(ppB[:, hsl], tanhpB[:, hsl], vvTB[:, hsl])
            nc.gpsimd.tensor_mul(qqB[:, hsl], tanhSB[:, hsl], vvTB[:, hsl])

            for Pl in range(4):
                P = 4 * H + Pl
                psl128 = slice(P * 128, (P + 1) * 128)
                qdst = (qa_ps if Pl < 2 else qb_ps)
                nc.tensor.matmul(
                    qdst[32 * (Pl % 2):32 * (Pl % 2) + 32, H * 128:(H + 1) * 128],
                    cs["bd_w2q"][:, :], ppB[:, psl128], start=True, stop=True)
                nc.tensor.matmul(t1e_ps[:, P * 16:(P + 1) * 16],
                                 t1preB[:, psl128], cs["selc"][:, :],
                                 start=True, stop=True)
                nc.tensor.matmul(qe_ps[:, P * 2:(P + 1) * 2],
                                 qqB[:, psl128], cs["onesc"][:, :],
                                 start=True, stop=True)

            # g-tilde for this H: psum [128, (Pl4, qs2, mn64)]
            gt_ps = gtps.tile([128, 512], f32, tag="gt_ps")
            for Pl in range(4):
                P = 4 * H + Pl
                gsl = slice(Pl * 128, (Pl + 1) * 128)
                nc.tensor.matmul(gt_ps[:, gsl], cs["ones1"][:1, :],
                                 cs["g0row10"][:1, :], start=True, stop=False)
                nc.tensor.matmul(gt_ps[:, gsl], tanhSB[:, P * 128:(P + 1) * 128],
                                 ident[:, :], is_transpose=True,
                                 start=False, stop=True)
            # LDL k=0 on this H (src = gt_ps), writes gA records for q in H-range
            q0 = 8 * H                      # first q of this H
            # invd0: diag mn=0
            nc.vector.reciprocal(
                invdA[:, q0:q0 + 8],
                gt_ps[:, :].rearrange("p (q mn) -> p q mn", mn=64)[:, :, 0])
            # wcol0: col0 rows 1..7 -> wcolA[(q in H), 7]
            wv = wcolA[:, 7 * q0:7 * (q0 + 8)].rearrange("p (q i) -> p q i", i=7)
            gtv = gt_ps[:, :].rearrange("p (q i j) -> p q i j", i=8, j=8)
            nc.vector.tensor_copy(wv[:, :, :], gtv[:, :, 1:8, 0])
            # l0 = wcol0 * invd0 -> gA col0
            gAv = gA[:, :].rearrange("p (q i j) -> p q i j", i=8, j=8)
            nc.vector.tensor_tensor(
                out=gAv[:, q0:q0 + 8, 1:8, 0], in0=wv[:, :, :],
                in1=invdA[:, q0:q0 + 8, None].broadcast_to([128, 8, 7]),
                op=OP.mult)
            # outer0 = l0_i * w0_j
            tv = tscrA[:, 49 * q0:49 * (q0 + 8)].rearrange(
                "p (q i j) -> p q i j", i=7, j=7)
            nc.vector.tensor_tensor(
                out=tv[:, :, :, :],
                in0=gAv[:, q0:q0 + 8, 1:8, 0:1].broadcast_to([128, 8, 7, 7]),
                in1=wv[:, :, None, :].broadcast_to([128, 8, 7, 7]),
                op=OP.mult)
            # sub0: gA rect rows1..7 = gt - outer
            nc.vector.tensor_tensor(
                out=gAv[:, q0:q0 + 8, 1:8, 1:8], in0=gtv[:, :, 1:8, 1:8],
                in1=tv[:, :, :, :], op=OP.subtract)

        # ---------- LDL k=1..7 on gA (all 32 q at once) ----------
        gAv = gA[:, :].rearrange("p (q i j) -> p q i j", i=8, j=8)
        wv7 = wcolA[:, :].rearrange("p (q i) -> p q i", i=7)
        tv7 = tscrA[:, :].rearrange("p (q i j) -> p q i j", i=7, j=7)
        for k in range(1, 7):
            m = 7 - k
            nc.vector.reciprocal(invdA[:, 32 * k:32 * (k + 1)], gAv[:, :, k, k])
            nc.vector.tensor_copy(wv7[:, :, :m], gAv[:, :, k + 1:8, k])
            nc.vector.tensor_tensor(
                out=gAv[:, :, k + 1:8, k], in0=wv7[:, :, :m],
                in1=invdA[:, 32 * k:32 * (k + 1), None].broadcast_to([128, 32, m]),
                op=OP.mult)
            nc.vector.tensor_tensor(
                out=tv7[:, :, :m, :m],
                in0=gAv[:, :, k + 1:8, k:k + 1].broadcast_to([128, 32, m, m]),
                in1=wv7[:, :, None, :m].broadcast_to([128, 32, m, m]),
                op=OP.mult)
            nc.vector.tensor_tensor(
                out=gAv[:, :, k + 1:8, k + 1:8], in0=gAv[:, :, k + 1:8, k + 1:8],
                in1=tv7[:, :, :m, :m], op=OP.subtract)
        nc.vector.reciprocal(invdA[:, 224:256], gAv[:, :, 7, 7])

        # ---------- q -> gpq -> T2E ----------
        gpqB = sb.tile([128, 512], f32, tag="gpqB")
        nc.vector.tensor_mul(gpqB[:64, :], gpuB[:64, :], qa_ps[:, :])
        nc.vector.tensor_mul(gpqB[64:, :], gpuB[64:, :], qb_ps[:, :])
        for H in range(4):
            nc.tensor.matmul(t2e_ps[:, H * 64:(H + 1) * 64],
                             gpqB[:, H * 128:(H + 1) * 128], cs["w1tc"][:, :],
                             start=True, stop=True)

        # ---------- Q, coefZ, z ----------
        qgA, qaA, czA, caA, dvA = (stile(t) for t in
                                   ["qgA", "qaA", "czA", "caA", "dvA"])
        nc.vector.tensor_copy(qgA[:, :], qg_ps[:, :])
        nc.vector.scalar_tensor_tensor(out=qaA[:, :], in0=qe_ps[:, :], scalar=0.1,
                                       in1=qgA[:, :], op0=OP.mult, op1=OP.add)
        nc.vector.tensor_mul(czA[:, :], qaA[:, :], kapA[:, :])
        nc.vector.tensor_mul(czA[:, :], czA[:, :], rinvA[:, :])
        # dv = sum_r dr0*v
        dvmA = sbA.tile([128, 256], f32, tag="dvmA")
        nc.vector.tensor_mul(dvmA[:, :], dr0_ps[:, :], vA[:, :])
        nc.vector.tensor_reduce(
            dvA[:, :], dvmA[:, :].rearrange("p (q r) -> p q r", r=8),
            axis=mybir.AxisListType.X, op=OP.add)
        nc.vector.scalar_tensor_tensor(out=caA[:, :], in0=kapA[:, :], scalar=2.0,
                                       in1=dvA[:, :], op0=OP.mult, op1=OP.mult)
        nc.vector.tensor_mul(caA[:, :], caA[:, :], rinvA[:, :])
        # z = 0.05*T1E - 0.5*T2E - cz*dr0
        t1s = sbA.tile([128, 256], f32, tag="t1s")
        zA = sbA.tile([128, 256], f32, tag="zA")
        nc.vector.tensor_tensor(
            out=t1s[:, :].rearrange("p (q r) -> p q r", r=8),
            in0=dr0_ps.rearrange("p (q r) -> p q r", r=8),
            in1=czA[:, :, None].broadcast_to([128, 32, 8]),
            op=OP.mult)
        nc.vector.scalar_tensor_tensor(out=zA[:, :], in0=t2e_ps[:, :], scalar=-0.5,
                                       in1=t1s[:, :], op0=OP.mult, op1=OP.subtract)
        nc.vector.scalar_tensor_tensor(out=zA[:, :], in0=t1e_ps[:, :], scalar=0.05,
                                       in1=zA[:, :], op0=OP.mult, op1=OP.add)

        # ---------- solve gA y = z ----------
        yv = zA[:, :].rearrange("p (q r) -> p q r", r=8)        # in-place y
        sv = sbA.tile([128, 224], f32, tag="solve_scr")
        svv = sv[:, :].rearrange("p (q i) -> p q i", i=7)
        for k in range(0, 7):
            m = 7 - k
            nc.vector.tensor_tensor(
                out=svv[:, :, :m], in0=gAv[:, :, k + 1:8, k],
                in1=yv[:, :, k:k + 1].broadcast_to([128, 32, m]), op=OP.mult)
            nc.vector.tensor_tensor(
                out=yv[:, :, k + 1:8], in0=yv[:, :, k + 1:8],
                in1=svv[:, :, :m], op=OP.subtract)
        nc.vector.tensor_tensor(
            out=yv[:, :, :],
            in0=yv[:, :, :],
            in1=invdA[:, :].rearrange("p (k q) -> p q k", q=32),
            op=OP.mult)
        for k in range(7, 0, -1):
            nc.vector.tensor_tensor(
                out=svv[:, :, :k], in0=gAv[:, :, k, 0:k],
                in1=yv[:, :, k:k + 1].broadcast_to([128, 32, k]), op=OP.mult)
            nc.vector.tensor_tensor(
                out=yv[:, :, 0:k], in0=yv[:, :, 0:k],
                in1=svv[:, :, :k], op=OP.subtract)

        # ---------- a = -coefA*v - 10*y ----------
        t3 = sbA.tile([128, 256], f32, tag="t3")
        nc.vector.tensor_tensor(
            out=t3[:, :].rearrange("p (q r) -> p q r", r=8),
            in0=vA[:, :].rearrange("p (q r) -> p q r", r=8),
            in1=caA[:, :, None].broadcast_to([128, 32, 8]),
            op=OP.mult)
        nc.vector.scalar_tensor_tensor(out=aA[:, :], in0=zA[:, :], scalar=-10.0,
                                       in1=t3[:, :], op0=OP.mult, op1=OP.subtract)

    # ================= chunk loop =================
    for c in range(NCHUNK):
        xA = sbA.tile([128, 256], f32, tag="xA")
        vA = sbA.tile([128, 256], f32, tag="vA")
        nc.sync.dma_start(out=xA[:, :], in_=dram_chunk(dram["x"], c))
        nc.sync.dma_start(out=vA[:, :], in_=dram_chunk(dram["v"], c))
        xT = transpose2(xA, "xT")
        vT = transpose2(vA, "vT")

        aA1 = sbA.tile([128, 256], f32, tag="aA1")
        emit_call(xT, vT, vA, aA1)

        vmidA = sbA.tile([128, 256], f32, tag="vmidA")
        nc.vector.scalar_tensor_tensor(out=vmidA[:, :], in0=aA1[:, :], scalar=0.05,
                                       in1=vA[:, :], op0=OP.mult, op1=OP.add)
        xnewA = sbA.tile([128, 256], f32, tag="xnewA")
        nc.vector.scalar_tensor_tensor(out=xnewA[:, :], in0=vmidA[:, :], scalar=0.1,
                                       in1=xA[:, :], op0=OP.mult, op1=OP.add)
        nc.sync.dma_start(out=dram_chunk(dram["x_new"], c), in_=xnewA[:, :])

        xmidT = sb.tile([64, 512], f32, tag="xmidT")
        nc.vector.scalar_tensor_tensor(out=xmidT[:, :], in0=vT[:, :], scalar=0.05,
                                       in1=xT[:, :], op0=OP.mult, op1=OP.add)
        vmidT = transpose2(vmidA, "vmidT")

        aA2 = sbA.tile([128, 256], f32, tag="aA2")
        emit_call(xmidT, vmidT, vmidA, aA2)

        vnewA = sbA.tile([128, 256], f32, tag="vnewA")
        nc.vector.scalar_tensor_tensor(out=vnewA[:, :], in0=aA2[:, :], scalar=0.1,
                                       in1=vA[:, :], op0=OP.mult, op1=OP.add)
        nc.sync.dma_start(out=dram_chunk(dram["v_new"], c), in_=vnewA[:, :])


def _build_module(consts, br2f):
    import concourse.bacc as bacc
    import concourse.mybir as mybir
    import concourse.tile as tile
    from contextlib import ExitStack

    f32 = mybir.dt.float32
    nc = bacc.Bacc("TRN2", target_bir_lowering=False, debug=False,
                   num_devices=NCORES)
    dram = {}
    dram["x"] = nc.dram_tensor("x", [NTOK, D], f32, kind="ExternalInput").ap()
    dram["v"] = nc.dram_tensor("v", [NTOK, D], f32, kind="ExternalInput").ap()
    for name, arr in consts.items():
        dram[name] = nc.dram_tensor(name, list(arr.shape), f32,
                                    kind="ExternalInput").ap()
    dram["x_new"] = nc.dram_tensor("x_new", [NTOK, D], f32,
                                   kind="ExternalOutput").ap()
    dram["v_new"] = nc.dram_tensor("v_new", [NTOK, D], f32,
                                   kind="ExternalOutput").ap()
    with tile.TileContext(nc) as tc:
        with ExitStack() as ctx:
            _emit(nc, tc, ctx, dram, br2f)
    nc.compile()
    return nc


def kernel(x, v, L, W1, b1, W2, b2, Wr1, br1, Wr2, br2):
    x = np.ascontiguousarray(np.asarray(x, dtype=np.float32))
    v = np.ascontiguousarray(np.asarray(v, dtype=np.float32))
    consts, br2f = _build_consts(L, W1, b1, W2, b2, Wr1, br1, Wr2, br2)
    nc = _build_module(consts, br2f)

    from concourse.bass_utils import run_bass_kernel_spmd
    in_maps = []
    for c in range(NCORES):
        m = {"x": np.ascontiguousarray(x[c]), "v": np.ascontiguousarray(v[c])}
        m.update(consts)
        in_maps.append(m)
    import os as _os
    trace = _os.environ.get("KERNEL_TRACE", "0") == "1"
    tdir = _os.environ.get("KERNEL_TRACE_DIR") or None
    if tdir:
        _os.makedirs(tdir, exist_ok=True)
    res = run_bass_kernel_spmd(nc, in_maps, core_ids=list(range(NCORES)),
                               trace=trace, tmpdir=tdir)
    global LAST_EXEC_TIME_NS, LAST_TRACE_PATH
    LAST_EXEC_TIME_NS = res.exec_time_ns
    LAST_TRACE_PATH = (res.instructions_and_trace[1]
                       if res.instructions_and_trace else None)
    x_new = np.stack([r["x_new"] for r in res.results]).astype(np.float32)
    v_new = np.stack([r["v_new"] for r in res.results]).astype(np.float32)
    return (x_new, v_new)

